# revision 22
# baseline (speedup 1.0000x reference)
"""Trainium2 Bass kernel for nn_Decoder_11613591569166.

The reference network (latent linear -> 3x [SkeletonUnpool -> masked
temporal conv (T=1) -> LeakyReLU]) collapses to a 3-layer dense MLP per
batch row, because with T=1 only the center conv tap contributes and the
unpool gathers are 0/1 linear maps that fold into the conv matrices.
Host side folds everything (incl. the de/re-normalization around the
quaternion normalize) into:

    h0 = lrelu(A0 @ x + c0)        A0: (128,256)
    h1 = lrelu(A1 @ h0 + c1)       A1: (104,128)
    y  = A2f @ h1 + c2f            A2f: (91,104)  rows 0..87 denormed motion
    ss = group4_sumsq(y[:88]); rs = ss^-1/2
    motion = y[:88] * expand(rs) * inv_s4 - m4/s4 ; disp = y[88:91]

Device: pure data parallel over 8 cores (16384 rows each). Per 512-row
tile: DMA in (natural layout) -> PE transpose to feature-major -> 3
matmul layers (fp32r, full PE rate) with fused bias+LeakyReLU on ACT ->
quat-normalize epilogue (PE group-sum + DVE pow(-0.5)) -> PE transpose
back to row-major -> DMA out.
"""

import os
import sys

import numpy as np

sys.path.insert(0, "/opt/trn_rl_repo")

# ---------------- problem constants (hardcoded; self-contained) -------------
B = 131072
LATENT = 256
N_CORES = 8
ROWS_PER_CORE = B // N_CORES          # 16384
TB = 512                               # batch rows per tile
NJ = 4                                 # 128-row chunks per tile
NT = ROWS_PER_CORE // TB               # 32 tiles per core
D0, D1, D2 = 128, 104, 91              # layer output dims (91 = 88 motion + 3 disp)
OUTW = 91
# packed-constants column map (fp32r pack)
CPR_A0, CPR_A1, CPR_A2 = 0, 256, 360
CPR_G4T, CPR_G4E, CPR_G4D = 451, 473, 564
CPR_ONESR, CPR_OBIAS, CPR_ONESW = 655, 1167, 1531
CPR_W = 1664
CPF_W = 132

PARENTS0 = np.array([-1, 0, 0, 0, 1, 2, 3, 4, 5, 6, 7, 8, 9, 9, 9, 12, 13,
                     14, 16, 17, 18, 19])
CH = [4, 8, 16, 32]
FACTOR = 2
NB_DIST = 2
K_T = 3
CTR = 1


# ---------------- skeleton topology (deterministic numpy) --------------------
def _pool(parents):
    n = len(parents)
    children = [[] for _ in range(n)]
    for j, p in enumerate(parents):
        if p >= 0:
            children[p].append(j)
    groups, gid, taken = [], [-1] * n, [False] * n
    for j in range(n):
        if taken[j]:
            continue
        g = [j]
        taken[j] = True
        if len(children[j]) == 1 and not taken[children[j][0]]:
            c = children[j][0]
            g.append(c)
            taken[c] = True
        for m in g:
            gid[m] = len(groups)
        groups.append(g)
    newp = np.array([-1 if parents[g[0]] < 0 else gid[parents[g[0]]]
                     for g in groups])
    return groups, newp


def _floyd(parents):
    n = len(parents)
    D = np.full((n, n), 10000)
    np.fill_diagonal(D, 0)
    for j, p in enumerate(parents):
        if p >= 0:
            D[j, p] = D[p, j] = 1
    for k in range(n):
        D = np.minimum(D, D[:, k:k + 1] + D[k:k + 1, :])
    return D


def _node_mask(parents, d):
    n = len(parents)
    M = np.zeros((n + 1, n + 1), bool)
    M[:n, :n] = _floyd(parents) <= d
    M[n, :n] = M[0, :n]
    M[:n, n] = M[:n, 0]
    M[n, n] = True
    return M


def _unpool_map(groups, n_fine):
    m = np.zeros(n_fine + 1, np.int32)
    for c, g in enumerate(groups):
        for f in g:
            m[f] = c
    m[n_fine] = len(groups)
    return m


_G01, _P1 = _pool(PARENTS0)
_G12, _P2 = _pool(_P1)
_G23, _P3 = _pool(_P2)
_UMAPS = [_unpool_map(_G23, len(_P2)), _unpool_map(_G12, len(_P1)),
          _unpool_map(_G01, len(PARENTS0))]
_LEVEL_PARENTS = [_P2, _P1, PARENTS0]


def _wmask_center(l):
    nm = _node_mask(_LEVEL_PARENTS[l], NB_DIST).astype(np.float32)
    cin, cout = CH[3 - l], CH[3 - l] // FACTOR
    return np.kron(nm, np.ones((cout, cin), np.float32))  # (N*cout, N*cin)


def _unpool_mat(umap, cin):
    nf = len(umap)
    ncoarse = int(umap.max()) + 1
    U = np.zeros((nf * cin, ncoarse * cin), np.float32)
    for f in range(nf):
        U[f * cin:(f + 1) * cin, umap[f] * cin:(umap[f] + 1) * cin] = np.eye(cin)
    return U


def _build_fused(input, mean_dqs, std_dqs, w_lat, b_lat, w0, b0, w1, b1, w2, b2):
    """Fold the whole network into 3 matrices + epilogue constants."""
    M = []
    for l, w in enumerate((w0, w1, w2)):
        M.append((np.asarray(w)[:, :, CTR] * _wmask_center(l)).astype(np.float64))
    U0 = _unpool_mat(_UMAPS[0], CH[3])
    U1 = _unpool_mat(_UMAPS[1], CH[2])
    U2 = _unpool_mat(_UMAPS[2], CH[1])
    w_lat = np.asarray(w_lat, np.float64)
    A0 = M[0] @ U0 @ w_lat
    c0 = M[0] @ U0 @ np.asarray(b_lat, np.float64) + np.asarray(b0, np.float64)
    A1 = M[1] @ U1
    c1 = np.asarray(b1, np.float64)
    A2 = M[2] @ U2
    c2 = np.asarray(b2, np.float64)
    m4 = np.asarray(mean_dqs, np.float64).reshape(-1, 8)[:, :4].reshape(-1)
    s4 = np.asarray(std_dqs, np.float64).reshape(-1, 8)[:, :4].reshape(-1)
    A2f = A2[:OUTW].copy()
    c2f = c2[:OUTW].copy()
    A2f[:88] *= s4[:, None]
    c2f[:88] = c2[:88] * s4 + m4
    inv_s4 = 1.0 / s4
    negb = m4 / s4
    f32 = lambda a: np.ascontiguousarray(a, np.float32)
    return dict(A0=f32(A0), c0=f32(c0), A1=f32(A1), c1=f32(c1),
                A2f=f32(A2f), c2f=f32(c2f), inv_s4=f32(inv_s4), negb=f32(negb))


def _pack_consts(F):
    """Pack all device constants into two (128, W) blocks (fp32r / fp32)."""
    cpr = np.zeros((128, CPR_W), np.float32)
    a0t = F["A0"].T                                  # (256, 128)
    cpr[:, CPR_A0:CPR_A0 + 128] = a0t[:128]
    cpr[:, CPR_A0 + 128:CPR_A0 + 256] = a0t[128:]
    cpr[:D0, CPR_A1:CPR_A1 + D1] = F["A1"].T
    cpr[:D1, CPR_A2:CPR_A2 + D2] = F["A2f"].T
    for i in range(88):
        cpr[i, CPR_G4T + i // 4] = 1.0
        cpr[i // 4, CPR_G4E + i] = F["inv_s4"][i]
    cpr[0, CPR_G4D + 88:CPR_G4D + OUTW] = 1.0
    cpr[0, CPR_ONESR:CPR_ONESR + TB] = 1.0
    neg = np.zeros(OUTW, np.float32)
    neg[:88] = -F["negb"]
    cpr[0, CPR_OBIAS:CPR_OBIAS + NJ * OUTW] = np.tile(neg, NJ)
    cpr[0, CPR_ONESW:CPR_ONESW + 128] = 1.0
    cpf = np.zeros((128, CPF_W), np.float32)
    cpf[:, :128] = np.eye(128, dtype=np.float32)
    cpf[:D0, 128] = F["c0"]
    cpf[:D1, 129] = F["c1"]
    cpf[:D2, 130] = F["c2f"]
    return cpr, cpf


# ---------------- Bass kernel ------------------------------------------------
_NC_CACHE = {}


def _build_bass(n_tiles=NT):
    """Build the per-core Bass module (SPMD: same program on all 8 cores)."""
    import concourse.bass as bass
    import concourse.bacc as bacc
    import concourse.tile as tile
    from concourse import mybir

    fp32 = mybir.dt.float32
    fp32r = mybir.dt.float32r
    AF = mybir.ActivationFunctionType
    ALU = mybir.AluOpType

    nc = bacc.Bacc("TRN2", name="decoder_mlp")

    n_rows = n_tiles * TB
    x_d = nc.dram_tensor("x", [n_rows, LATENT], fp32, kind="ExternalInput")
    # all fp32r consts packed into one tensor -> one DMA -> one semaphore
    cpr_d = nc.dram_tensor("cpr", [128, CPR_W], fp32r, kind="ExternalInput")
    # all fp32 consts (identity + per-row biases) in a second tensor
    cpf_d = nc.dram_tensor("cpf", [128, CPF_W], fp32, kind="ExternalInput")
    out_d = nc.dram_tensor("out", [n_rows, OUTW], fp32,
                           kind="ExternalOutput")

    with tile.TileContext(nc) as tc:
        with tc.tile_pool(name="consts", bufs=1) as consts, \
             tc.tile_pool(name="xin", bufs=3) as xin_p, \
             tc.tile_pool(name="xt", bufs=2) as xt_p, \
             tc.tile_pool(name="hid", bufs=2) as hid_p, \
             tc.tile_pool(name="epi", bufs=2) as epi_p, \
             tc.tile_pool(name="obuf", bufs=3) as obuf_p, \
             tc.tile_pool(name="ps_t", bufs=2, space="PSUM") as ps_t, \
             tc.tile_pool(name="ps_mm", bufs=2, space="PSUM") as ps_mm, \
             tc.tile_pool(name="ps_ss", bufs=1, space="PSUM") as ps_ss, \
             tc.tile_pool(name="ps_e", bufs=1, space="PSUM") as ps_e, \
             tc.tile_pool(name="ps_o", bufs=2, space="PSUM") as ps_o:

            # ---- constants: 2 packed DMAs, then tiny per-engine warmups ----
            cpr = consts.tile([128, CPR_W], fp32r)
            nc.sync.dma_start(out=cpr, in_=cpr_d[:, :])
            cpf = consts.tile([128, CPF_W], fp32)
            nc.sync.dma_start(out=cpf, in_=cpf_d[:, :])

            a0t_s = cpr[:, 0:256].rearrange("p (k m) -> p k m", k=2)
            a1t_s = cpr[:D0, CPR_A1:CPR_A1 + D1]
            a2t_s = cpr[:D1, CPR_A2:CPR_A2 + D2]
            g4t_s = cpr[:88, CPR_G4T:CPR_G4T + 22]
            g4e_s = cpr[:22, CPR_G4E:CPR_G4E + OUTW]
            g4d_s = cpr[0:1, CPR_G4D:CPR_G4D + OUTW]
            onesr_s = cpr[0:1, CPR_ONESR:CPR_ONESR + TB]
            obias_s = cpr[0:1, CPR_OBIAS:CPR_OBIAS + NJ * OUTW]
            ones_s = cpr[0:1, CPR_ONESW:CPR_ONESW + 128]
            ident = cpf[:, 0:128]
            c0_s = cpf[:D0, 128:129]
            c1_s = cpf[:D1, 129:130]
            c2_s = cpf[:D2, 130:131]

            # warmups: advance each engine's clock past the const DMAs so
            # steady-state instructions need at most one fresh wait (the
            # fp32r matmul HW instruction has a single sync-wait slot)
            warm_ps = ps_o.tile([128, 128], fp32, tag="o")
            nc.tensor.transpose(warm_ps, ident, ident)
            warm_ps2 = ps_o.tile([128, NJ * OUTW], fp32, tag="o")
            nc.tensor.matmul(warm_ps2[:, 0:NJ * OUTW], ones_s, obias_s,
                             start=True, stop=True)
            warm_sb = consts.tile([1, 1], fp32)
            nc.scalar.copy(warm_sb, cpf[0:1, 128:129])
            warm_sb2 = consts.tile([1, 1], fp32)
            nc.vector.tensor_copy(warm_sb2, cpr[0:1, 0:1])

            for it in range(n_tiles):
                b0r = it * TB
                # ---- load 512 rows in natural layout ----
                x_nat = xin_p.tile([128, NJ, LATENT], fp32)
                nc.sync.dma_start(
                    out=x_nat,
                    in_=x_d[b0r:b0r + TB, :].rearrange("(j p) f -> p j f", p=128),
                )

                # ---- transpose to feature-major: xt[f][128, TB] ----
                xt = xt_p.tile([128, 2, TB], fp32r)
                for f in range(2):
                    t_ps = ps_t.tile([128, TB], fp32, tag="t")
                    for j in range(NJ):
                        nc.tensor.transpose(
                            t_ps[:, j * 128:(j + 1) * 128],
                            x_nat[:, j, f * 128:(f + 1) * 128],
                            ident,
                        )
                    eng = nc.scalar if f == 0 else nc.vector
                    if f == 0:
                        nc.scalar.activation(xt[:, f, :], t_ps, AF.Copy)
                    else:
                        nc.vector.tensor_copy(xt[:, f, :], t_ps)

                # ---- L0: h0 = lrelu(A0 @ xT + c0) ----
                h0_ps = ps_mm.tile([D0, TB], fp32, tag="mm")
                nc.tensor.matmul(h0_ps, a0t_s[:, 0, :], xt[:, 0, :],
                                 start=True, stop=False)
                nc.tensor.matmul(h0_ps, a0t_s[:, 1, :], xt[:, 1, :],
                                 start=False, stop=True)
                h0_s = hid_p.tile([D0, TB], fp32r, tag="h0")
                nc.scalar.activation(h0_s, h0_ps, AF.Prelu, bias=c0_s, alpha=0.2)

                # ---- L1 ----
                h1_ps_full = ps_mm.tile([128, TB], fp32, tag="mm")
                h1_ps = h1_ps_full[:D1, :]
                nc.tensor.matmul(h1_ps, a1t_s, h0_s, start=True, stop=True)
                h1_s = hid_p.tile([D1, TB], fp32r, tag="h1")
                nc.scalar.activation(h1_s, h1_ps, AF.Prelu, bias=c1_s, alpha=0.2)

                # ---- L2 (rows 0..87 denormed motion, 88..90 disp) ----
                y_ps_full = ps_mm.tile([128, TB], fp32, tag="mm")
                y_ps = y_ps_full[:D2, :]
                nc.tensor.matmul(y_ps, a2t_s[:, :D2], h1_s,
                                 start=True, stop=True)
                ys = epi_p.tile([D2, TB], fp32, tag="ys")
                nc.scalar.activation(ys, y_ps, AF.Identity, bias=c2_s)
                sq = epi_p.tile([88, TB], fp32r, tag="sq")
                nc.scalar.activation(sq, y_ps[:88, :], AF.Square, bias=c2_s[:88, :])

                # ---- group-of-4 sum of squares -> rs = ss^-0.5 ----
                ss_ps = ps_ss.tile([22, TB], fp32, tag="ss")
                nc.tensor.matmul(ss_ps, g4t_s, sq, start=True, stop=True)
                srt = epi_p.tile([22, TB], fp32, tag="srt")
                nc.scalar.activation(srt, ss_ps, AF.Sqrt)
                rs = epi_p.tile([22, TB], fp32r, tag="rs")
                with nc.allow_low_precision(reason="fp32r is full-width"):
                    nc.vector.reciprocal(rs, srt)

                # ---- expand rs (incl inv_s4 fold); disp rows get 1.0 ----
                e_ps = ps_e.tile([D2, TB], fp32, tag="e")
                nc.tensor.matmul(e_ps, g4e_s, rs, start=True, stop=False,
                                 skip_group_check=True)
                nc.tensor.matmul(e_ps, g4d_s, onesr_s, start=False, stop=True,
                                 skip_group_check=True)
                yse = epi_p.tile([D2, TB], fp32, tag="yse")
                nc.vector.tensor_tensor(yse, ys, e_ps, op=ALU.mult)

                # ---- transpose back to row-major + add -m4/s4 bias ----
                o_ps = ps_o.tile([128, NJ * OUTW], fp32, tag="o")
                nc.tensor.matmul(o_ps, ones_s, obias_s, start=True, stop=False,
                                 skip_group_check=True)
                for j in range(NJ):
                    nc.tensor.matmul(
                        o_ps[:, j * OUTW:(j + 1) * OUTW],
                        yse[:, j * 128:(j + 1) * 128],
                        ident[:D2, :D2],
                        is_transpose=True,
                        start=False, stop=(j == NJ - 1),
                        skip_group_check=True,
                    )
                obuf = obuf_p.tile([128, NJ * OUTW], fp32)
                nc.vector.tensor_copy(obuf, o_ps)
                nc.sync.dma_start(
                    out=out_d[b0r:b0r + TB, :].rearrange("(j p) f -> p j f", p=128),
                    in_=obuf.rearrange("p (j f) -> p j f", j=NJ),
                )

    nc.compile()
    return nc


def _get_nc():
    if "nc" not in _NC_CACHE:
        _NC_CACHE["nc"] = _build_bass()
    return _NC_CACHE["nc"]


TRACE = False
LAST_EXEC_NS = None


def kernel(input, mean_dqs, std_dqs, w_lat, b_lat, w0, b0, w1, b1, w2, b2):
    global LAST_EXEC_NS
    from concourse.bass_utils import run_bass_kernel_spmd

    F = _build_fused(input, mean_dqs, std_dqs, w_lat, b_lat,
                     w0, b0, w1, b1, w2, b2)

    cpr, cpf = _pack_consts(F)
    x_full = np.ascontiguousarray(np.asarray(input, np.float32))
    in_maps = []
    for c in range(N_CORES):
        in_maps.append({"x": x_full[c * ROWS_PER_CORE:(c + 1) * ROWS_PER_CORE],
                        "cpr": cpr, "cpf": cpf})

    nc = _get_nc()
    res = run_bass_kernel_spmd(nc, in_maps, core_ids=list(range(N_CORES)),
                               trace=TRACE)
    if res.exec_time_ns is not None:
        LAST_EXEC_NS = res.exec_time_ns
    out = np.concatenate([res.results[c]["out"] for c in range(N_CORES)], axis=0)
    motion = np.ascontiguousarray(out[:, :88])[:, :, None]
    disp = np.ascontiguousarray(out[:, 88:OUTW])[:, :, None]
    return motion.astype(np.float32), disp.astype(np.float32)
